# revision 17
# baseline (speedup 1.0000x reference)
"""LoRA Linear kernel for Trainium2, 8 NeuronCores, data-parallel over tokens.

out = x @ W^T + bias + 2.0 * (x @ A^T) @ B^T
  x: [4, 2048, 4096] f32, W: [4096, 4096], bias: [4096], A: [16, 4096], B: [4096, 16]

Strategy:
  - Host folds the rank-16 LoRA update into the weight: W' = W + 2*B@A, so the
    device does a single dense GEMM + bias. Bias is fused into the PSUM->SBUF
    drain on the vector engine (tensor_scalar_add with a per-partition scalar).
  - Flatten tokens (8192) and shard 1024 tokens per core (pure data parallel,
    no collectives; gather on host).
  - bf16 operands and output (fp32 PSUM accumulation): same 1 cycle/row PE
    rate as f32r, half the HBM traffic and SBUF footprint.
  - Host pre-arranges operands so every DMA is contiguous per partition:
      xt [128, 32, 1024]: xt[p, k, m] = x_shard^T[k*128+p, m]
      wt [32, 128, 32, 128]: wt[oi, p, k, o] = W'^T[k*128+p, oi*128+o]
  - Each core computes out^T [4096, 1024] in [o=128, m=1024] PSUM pair-tiles
    (2 banks), accumulating 2x32 K=128 matmuls (W' stationary, x^T moving,
    N=512). One DVE drain + one store per o-tile.
  - Ramp: the first NRAMP o-tiles run k-outer with staggered starts (tile t
    enters at k=STRIDE*t, wraps around), so the PE consumes x chunks at the
    DMA frontier instead of waiting for the full x^T load.
  - Single SBUF pool + single PSUM pool: each pool release costs a 5-engine
    barrier round in the epilogue (~0.5 us each).
"""

import sys
from contextlib import ExitStack

import numpy as np

sys.path.insert(0, "/opt/trn_rl_repo")

import concourse.bacc as bacc  # noqa: E402
import concourse.bass as bass  # noqa: E402
import concourse.mybir as mybir  # noqa: E402
import concourse.tile as tile  # noqa: E402
from concourse.bass import ts  # noqa: E402
from concourse.bass_utils import run_bass_kernel_spmd  # noqa: E402

from ml_dtypes import bfloat16  # noqa: E402

P = 128
B_DIM, S_DIM = 4, 2048
D = 4096          # in_features (contraction)
O = 4096          # out_features
R = 16            # lora rank
SCALING = 2.0     # alpha / rank = 32/16
NCORES = 8
M = (B_DIM * S_DIM) // NCORES   # tokens per core = 1024
KD = D // P       # 32 contraction tiles
MC = 512          # moving free dim per matmul (one PSUM bank of fp32)
NMC = M // MC     # 2 m-chunks
NO = O // P       # 32 output-feature tiles
KC = 2            # k-tiles per x-chunk DMA -> 16 chunks
NRAMP = 4         # o-tiles k-outer-interleaved during the x-load ramp
STRIDE = 2        # k-slot stagger between consecutive ramp tiles

BF = mybir.dt.bfloat16
F32 = mybir.dt.float32


def build_program() -> bass.Bass:
    # Bacc (not plain Bass): its compile() pipeline moves extra matmul waits
    # onto LDWEIGHTS and splits any remainder via event semaphores.
    nc = bacc.Bacc()
    xt = nc.dram_tensor("xt", [P, KD, M], BF, kind="ExternalInput")
    wt = nc.dram_tensor("wt", [NO, P, KD, P], BF, kind="ExternalInput")
    bs = nc.dram_tensor("bs", [P, NO], F32, kind="ExternalInput")
    outT = nc.dram_tensor("outT", [O, M], BF, kind="ExternalOutput")

    with ExitStack() as ctx:
        tc = ctx.enter_context(tile.TileContext(nc))
        pool = ctx.enter_context(tc.tile_pool(name="sb", bufs=1))
        ps_pool = ctx.enter_context(tc.tile_pool(name="psp", bufs=4, space="PSUM"))

        xt_sb = pool.tile([P, KD, M], BF)
        bias_sb = pool.tile([P, NO], F32)

        def w_load(oi, split=1):
            wt_sb = pool.tile([P, KD, P], BF, name="wt_sb", bufs=4)
            kc = KD // split
            for h in range(split):
                nc.sync.dma_start(
                    wt_sb[:, ts(h, kc), :], wt[oi, :, ts(h, kc), :]
                )
            return wt_sb

        # W stream on the sync HWDGE ring; W0 split so its first k-tiles
        # land early (the first matmul needs only W0[k<8] + x-chunk 0).
        ramp_wt = [w_load(t, split=(4 if t == 0 else 1)) for t in range(NRAMP)]
        nc.scalar.dma_start(bias_sb[:], bs[:])
        # x^T split along the contraction dim on the scalar HWDGE ring:
        # ramp matmuls for k-chunk c only wait on chunk c.
        for c in range(KD // KC):
            nc.scalar.dma_start(xt_sb[:, ts(c, KC), :], xt[:, ts(c, KC), :])

        def drain(oi, ps):
            ot = pool.tile([P, M], BF, name="ot", bufs=4)
            nc.vector.tensor_scalar_add(ot[:], ps[:], bias_sb[:, oi : oi + 1])
            nc.scalar.dma_start(outT[ts(oi, P), :], ot[:])

        def mm_pair(ps, wt_sb, k, start, stop):
            for mi in range(NMC):
                nc.tensor.matmul(
                    ps[:, ts(mi, MC)],
                    lhsT=wt_sb[:, k, :],
                    rhs=xt_sb[:, k, ts(mi, MC)],
                    start=start,
                    stop=stop,
                    skip_group_check=True,
                )

        # Ramp: NRAMP o-tiles advance together along the x-chunk frontier;
        # tile t joins at k=STRIDE*t and wraps to finish its first k's last.
        # Same "ps" ring tag as the steady loop: 4 pair-tiles = all 8 banks;
        # steady allocations wrap the ring and wait on ramp drains naturally.
        ramp_ps = [
            ps_pool.tile([P, M], F32, name="ps") for _ in range(NRAMP)
        ]

        # Warm-up: dummy matmuls on a zeroed tile with no DMA deps keep the
        # PE busy from ~8 us (right after the memset) so the HAM clock-gate
        # is at 8/8 (2.4 GHz) when the first real matmul's data lands
        # (~12 us); the first real start=True clears the scratch bank.
        NWARM = 16
        dummy = pool.tile([P, MC], BF, name="dummy")
        nc.vector.memset(dummy[:], 0.0)
        for i in range(NWARM):
            nc.tensor.matmul(
                ramp_ps[0][:, 0:MC],
                lhsT=dummy[:, 0:P],
                rhs=dummy[:],
                start=(i == 0),
                stop=False,
                skip_group_check=True,
            )
        for s in range(KD + STRIDE * (NRAMP - 1)):
            for t in range(NRAMP):
                if s < STRIDE * t or s >= STRIDE * t + KD:
                    continue
                k = s if s < KD else s - KD
                mm_pair(
                    ramp_ps[t],
                    ramp_wt[t],
                    k,
                    start=(s == STRIDE * t),
                    stop=(s == STRIDE * t + KD - 1),
                )
            for t in range(NRAMP):
                if s == STRIDE * t + KD - 1:
                    drain(t, ramp_ps[t])

        # Steady state: one o-tile at a time, W blocks prefetched 3 deep.
        # The last tile drains per m-chunk (mi=0 drain overlaps mi=1's
        # matmuls; final store is half-size) to shorten the tail.
        for oi in range(NRAMP, NO):
            wt_sb = w_load(oi)
            last = oi == NO - 1
            ps = ps_pool.tile([P, M], F32, name="ps")
            for mi in range(NMC):
                for k in range(KD):
                    nc.tensor.matmul(
                        ps[:, ts(mi, MC)],
                        lhsT=wt_sb[:, k, :],
                        rhs=xt_sb[:, k, ts(mi, MC)],
                        start=(k == 0),
                        stop=(k == KD - 1),
                    )
                if last:
                    ot = pool.tile([P, MC], BF, name=f"lot{mi}", bufs=1)
                    nc.vector.tensor_scalar_add(
                        ot[:], ps[:, ts(mi, MC)], bias_sb[:, oi : oi + 1]
                    )
                    eng = nc.scalar if mi == 0 else nc.sync
                    eng.dma_start(outT[ts(oi, P), ts(mi, MC)], ot[:])
            if not last:
                drain(oi, ps)
    nc.compile()
    return nc


def prepare_in_maps(inputs, weight, bias, lora_a, lora_b):
    w_eff = np.asarray(weight, dtype=np.float32) + SCALING * (
        np.asarray(lora_b, dtype=np.float32) @ np.asarray(lora_a, dtype=np.float32)
    )
    # wt[oi, p, k, o] = W'^T[k*128+p, oi*128+o]; contiguous 8 KiB/partition blocks
    wt = np.ascontiguousarray(
        w_eff.T.reshape(KD, P, NO, P).transpose(2, 1, 0, 3)
    ).astype(bfloat16)
    bs = np.ascontiguousarray(np.asarray(bias, dtype=np.float32).reshape(NO, P).T)
    x = np.asarray(inputs, dtype=np.float32).reshape(B_DIM * S_DIM, D)
    in_maps = []
    for c in range(NCORES):
        # xt[p, k, m] = x_shard^T[k*128+p, m]; contiguous 64 KiB/partition
        xt_c = np.ascontiguousarray(
            x[c * M : (c + 1) * M].T.reshape(KD, P, M).transpose(1, 0, 2)
        ).astype(bfloat16)
        in_maps.append({"xt": xt_c, "wt": wt, "bs": bs})
    return in_maps


def run(inputs, weight, bias, lora_a, lora_b, trace=False):
    nc = build_program()
    in_maps = prepare_in_maps(inputs, weight, bias, lora_a, lora_b)
    res = run_bass_kernel_spmd(nc, in_maps, list(range(NCORES)), trace=trace)
    shards = [
        np.asarray(res.results[c]["outT"]).astype(np.float32).T
        for c in range(NCORES)
    ]
    out = np.concatenate(shards, axis=0).reshape(B_DIM, S_DIM, O)
    return np.ascontiguousarray(out, dtype=np.float32), res


def kernel(inputs, weight, bias, lora_a, lora_b):
    out, _ = run(inputs, weight, bias, lora_a, lora_b, trace=False)
    return out
